# revision 40
# baseline (speedup 1.0000x reference)
"""Trainium2 Bass kernel for nn_AlignModule_full (8 NeuronCores, data-parallel).

Reference computation: two 1x1 convs -> concat -> 3x3 conv + BN + ReLU ->
3x3 conv -> flow -> bilinear grid_sample warp of t2_pred, where output
channel (n, ch) is warped with flow[(3n+ch) % 4] (torch flow.repeat
semantics faithfully ported by the reference).

Sharding: core c = (q, h), q = c//2 flow batch, h = c%2 row half.
Each core computes flow(q, rows 64h..64h+64) from batch-q features, then
warps the 19 (n, ch) images with (3n+ch)%4 == q for its row half, using
only its own flow. Zero cross-core communication.

v2 layout notes:
- feature loads in 4-row chunks (big DMA descriptors) on sync+vector queues
- phase 3 (3x3 conv 64->2) runs 3 col-tiled x 2 row-tiled concurrent
  matmuls via tile_position (M=2 only needs a 32-wide PE strip)
- flow goes to DRAM row-major bf16 (2KB descriptors), comes back through
  the DMA transpose XBAR into CL2 layout: partition p = 16G+m,
  free f = 64w+r, pixel (row 64h+r, col 128w+16G+m)
- output is dumped as raw [call, slot, 128, 1024] tiles; host unshards
"""
import sys

for _p in ('/opt/trn_rl_repo',):
    if _p not in sys.path:
        sys.path.append(_p)

import numpy as np
import ml_dtypes

import concourse.bass as bass
import concourse.bacc as bacc
import concourse.mybir as mybir
import concourse.tile as tile

F32 = mybir.dt.float32
BF16 = mybir.dt.bfloat16
I16 = mybir.dt.int16
AF = mybir.ActivationFunctionType
ALU = mybir.AluOpType

H, W, CIN, T, CCLS, NB = 128, 256, 256, 64, 19, 4
SLAB_R = 68          # feature slab rows
WS = 258             # padded width for t/x buffers
XR = 66              # x rows total
XH = 36              # x rows per partition-half (A: 0..36, B: 30..66)
YS, XS = 76, 26      # gather slab rows/cols per (group, call=col-half)
LNUM = YS * XS       # base positions per partition
DCH = 8              # interleave chunk: 2 slots x (2x2 patch)
NIDX = 1024          # gather indices per group per call
ROWB = 6             # slab row margin before first output row of the call
COLB = 5             # slab col margin before group col block

BF = ml_dtypes.bfloat16


def img_list(q):
    return [(n, ch) for n in range(NB) for ch in range(CCLS)
            if (3 * n + ch) % 4 == q]


def build_nc():
    nc = bacc.Bacc(None, target_bir_lowering=False, debug=False)
    P = nc.declare_dram_parameter
    f1_d = P("f1", [2, 128, SLAB_R, W], BF16, isOutput=False)
    f2_d = P("f2", [2, 128, SLAB_R, W], BF16, isOutput=False)
    wd_d = P("wd", [128, 2, 2, T], BF16, isOutput=False)
    wf1_d = P("wf1", [128, 9, T], BF16, isOutput=False)
    wf2_d = P("wf2", [128, 2, 9, 2], BF16, isOutput=False)
    bn_d = P("bn", [128, 2, 1], F32, isOutput=False)
    mask_d = P("mask", [128, 2, 1], F32, isOutput=False)
    bx_d = P("bx", [128, 128], F32, isOutput=False)
    by_d = P("by", [128, 128], F32, isOutput=False)
    ylo_d = P("ylo", [128, 128], F32, isOutput=False)
    yhi_d = P("yhi", [128, 128], F32, isOutput=False)
    xlo_d = P("xlo", [128, 128], F32, isOutput=False)
    xhi_d = P("xhi", [128, 128], F32, isOutput=False)
    emat_d = P("emat", [8, 128], BF16, isOutput=False)
    dsrc_d = P("dsrc", [2, 128, LNUM * DCH], BF16, isOutput=False)
    out_d = P("out", [2, 2, 128, NIDX], F32, isOutput=True)

    flow_rm = nc.dram_tensor("flow_rm", [2, 64, W], BF16)   # row-major bf16
    w_dram = nc.dram_tensor("w_dram", [4 * 128 * 128], BF16)

    NCH = 9  # feature chunks of 8 rows (last chunk 4)

    with tile.TileContext(nc) as tc:
        with (
            tc.tile_pool(name="stream", bufs=3) as sp,
            tc.tile_pool(name="big", bufs=1) as bp,
            tc.tile_pool(name="psA", bufs=2, space="PSUM") as pp,
        ):
            # ---- constants ----
            wd_s = bp.tile([128, 2, 2, T], BF16, tag="wd")
            wf1_s = bp.tile([128, 9, T], BF16, tag="wf1")
            wf2_s = bp.tile([128, 2, 9, 2], BF16, tag="wf2")
            bn_s = bp.tile([128, 2, 1], F32, tag="bn")
            mask_s = bp.tile([128, 2, 1], F32, tag="mask")
            xlo_s = bp.tile([128, 128], F32, tag="xlo")
            xhi_s = bp.tile([128, 128], F32, tag="xhi")
            emat_s = bp.tile([8, 128], BF16, tag="emat")
            bx_s = bp.tile([128, 128], F32, tag="bx")
            by_s = bp.tile([128, 128], F32, tag="by")
            ylo_s = bp.tile([128, 128], F32, tag="ylo")
            yhi_s = bp.tile([128, 128], F32, tag="yhi")
            nc.sync.dma_start(wd_s[:], wd_d[:])

            # ---- big shared tiles ----
            t_cat = bp.tile([128, SLAB_R * WS], BF16, tag="tcat_gat")
            dsrc = bp.tile([128, LNUM * DCH], BF16, tag="dsrc")
            dsrc2 = bp.tile([128, LNUM * DCH], BF16, tag="dsrc2")
            x_sb = bp.tile([128, XH * WS], BF16, tag="x_w4")

            t3 = t_cat[:].rearrange("p (r c) -> p r c", r=SLAB_R, c=WS)
            nc.vector.memset(t3[:, :, 0:1], 0.0)
            nc.vector.memset(t3[:, :, 257:258], 0.0)

            # ---- feature chunk loads: 4 rows per chunk, 2 DMAs each ----
            fch = {}

            def load_chunk(k):
                c1 = sp.tile([128, 2, 8, W], BF16, tag="fc1", name=f"fc1_{k}",
                             bufs=2)
                c2 = sp.tile([128, 2, 8, W], BF16, tag="fc2", name=f"fc2_{k}",
                             bufs=2)
                r0 = 8 * k
                nr = min(8, SLAB_R - r0)
                qa, qb = (nc.sync, nc.scalar) if k % 2 == 0 else (nc.scalar, nc.sync)
                qa.dma_start(c1[:, 0, 0:nr], f1_d[0, :, r0:r0 + nr, :])
                qb.dma_start(c1[:, 1, 0:nr], f1_d[1, :, r0:r0 + nr, :])
                qa.dma_start(c2[:, 0, 0:nr], f2_d[0, :, r0:r0 + nr, :])
                qb.dma_start(c2[:, 1, 0:nr], f2_d[1, :, r0:r0 + nr, :])
                fch[k] = (c1, c2)

            for k in range(2):
                load_chunk(k)
            for t_, d_ in ((wf1_s, wf1_d), (bn_s, bn_d)):
                nc.sync.dma_start(t_[:], d_[:])
            for t_, d_ in ((wf2_s, wf2_d),
                           (mask_s, mask_d), (xlo_s, xlo_d),
                           (xhi_s, xhi_d),
                           (emat_s, emat_d), (bx_s, bx_d), (by_s, by_d),
                           (ylo_s, ylo_d), (yhi_s, yhi_d)):
                nc.scalar.dma_start(t_[:], d_[:])

            # warm the gpsimd ext-isa gather lib early so the real gathers
            # (and nothing else on gpsimd) don't pay the IRAM load later
            widx = bp.tile([128, 4], I16, tag="widx")
            wout = bp.tile([128, 128], BF16, tag="wout")
            wsrc = bp.tile([128, 256], BF16, tag="wsrc")
            nc.vector.memset(widx[:], 0)
            nc.vector.memset(wsrc[:], 0.0)
            nc.gpsimd.ap_gather(
                wout[:],
                wsrc[:].rearrange("p (n d) -> p n d", d=8),
                widx[:, 0:1],
                channels=128, num_elems=32, d=8, num_idxs=16)

            # ---- phases 1+2 interleaved: 1x1 convs feed 3x3 conv ----
            def p1_tile(it):
                r0 = 2 * it
                k, lr = it // 4, 2 * (it % 4)
                if lr == 0 and (k + 2) < NCH and (k + 2) not in fch:
                    load_chunk(k + 2)
                c1, c2 = fch[k]
                ps = pp.tile([128, 2 * W], F32, tag="pst", name="pst")
                for ck in range(2):
                    nc.tensor.matmul(ps[0:T, :], wd_s[:, 0, ck, :],
                                     c1[:, ck, lr:lr + 2, :],
                                     start=(ck == 0), stop=(ck == 1))
                for ck in range(2):
                    nc.tensor.matmul(ps[T:128, :], wd_s[:, 1, ck, :],
                                     c2[:, ck, lr:lr + 2, :],
                                     start=(ck == 0), stop=(ck == 1))
                dst = bass.AP(tensor=t_cat.tensor, offset=r0 * WS + 1,
                              ap=[[SLAB_R * WS, 128], [WS, 2], [1, W]])
                nc.vector.tensor_copy(dst, ps[:].rearrange("p (r c) -> p r c",
                                                           r=2, c=W))

            x3 = x_sb[:].rearrange("p (r c) -> p r c", r=XH, c=WS)
            nc.vector.memset(x3[:, :, 0:1], 0.0)
            nc.vector.memset(x3[:, :, 257:258], 0.0)

            def p2_iter(it):
                jA = 2 * it
                jB = 30 + 2 * it
                ps = pp.tile([128, 2 * W], F32, tag="psx", name="psx")
                for tap in range(9):
                    dy, dx = tap // 3, tap % 3
                    rhsA = bass.AP(tensor=t_cat.tensor,
                                   offset=(jA + dy) * WS + dx,
                                   ap=[[SLAB_R * WS, 128], [WS, 2], [1, W]])
                    rhsB = bass.AP(tensor=t_cat.tensor,
                                   offset=(jB + dy) * WS + dx,
                                   ap=[[SLAB_R * WS, 128], [WS, 2], [1, W]])
                    nc.tensor.matmul(ps[0:T, :], wf1_s[:, tap, :], rhsA,
                                     start=(tap == 0), stop=(tap == 8),
                                     tile_position=(0, 0),
                                     skip_group_check=True)
                    nc.tensor.matmul(ps[T:128, :], wf1_s[:, tap, :], rhsB,
                                     start=(tap == 0), stop=(tap == 8),
                                     tile_position=(0, 64),
                                     skip_group_check=True)
                dstA = bass.AP(tensor=x_sb.tensor, offset=jA * WS + 1,
                               ap=[[XH * WS, T], [WS, 2], [1, W]])
                dstB = bass.AP(tensor=x_sb.tensor,
                               offset=T * (XH * WS) + jA * WS + 1,
                               ap=[[XH * WS, T], [WS, 2], [1, W]])
                nc.scalar.activation(dstA,
                                     ps[0:T].rearrange("p (r c) -> p r c", r=2, c=W),
                                     AF.Relu, bias=bn_s[0:T, 1], scale=bn_s[0:T, 0])
                nc.scalar.activation(dstB,
                                     ps[T:128].rearrange("p (r c) -> p r c", r=2, c=W),
                                     AF.Relu, bias=bn_s[T:128, 1], scale=bn_s[T:128, 0])

            for it in range(34):
                p1_tile(it)
                if it >= 16:
                    p2_iter(it - 16)
            nc.vector.tensor_scalar_mul(x3[0:T, 0, :], x3[0:T, 0, :], mask_s[0:T, 0])
            nc.vector.tensor_scalar_mul(x3[T:128, 35, :], x3[T:128, 35, :],
                                        mask_s[T:128, 1])

            # dsrc loads on sync/scalar (keep gpsimd gather-only so its
            # ext-isa lib stays resident).  Gated so the scheduler doesn't
            # hoist these 2MB transfers into the feature-load window.
            with tc.tile_wait_until(0.14):
                nc.scalar.dma_start(dsrc[:, 0:LNUM * DCH // 2],
                                    dsrc_d[0, :, 0:LNUM * DCH // 2])
                nc.scalar.dma_start(dsrc[:, LNUM * DCH // 2:],
                                    dsrc_d[0, :, LNUM * DCH // 2:])
            with tc.tile_wait_until(0.17):
                nc.scalar.dma_start(dsrc2[:, 0:LNUM * DCH // 2],
                                    dsrc_d[1, :, 0:LNUM * DCH // 2])
                nc.scalar.dma_start(dsrc2[:, LNUM * DCH // 2:],
                                    dsrc_d[1, :, LNUM * DCH // 2:])

            # ---- phase 3: 3x3 conv 64->2, groups of 3 col-tiled chunks ----
            # chunk it -> flow rows (2it, 2it+1); it 0..15 from x half A
            # (row tile 0), it 16..31 from half B (row tile 64).
            for g in range(11):
                its = [i for i in (3 * g, 3 * g + 1, 3 * g + 2) if i < 32]
                psf = pp.tile([128, 2 * W], F32, tag="psf", name="psf", bufs=3)
                for tap in range(9):
                    dy, dx = tap // 3, tap % 3
                    for ci, it in enumerate(its):
                        i0 = 2 * it
                        hf = it // 16
                        base = i0 + dy - 30 * hf
                        rhs = bass.AP(tensor=x_sb.tensor,
                                      offset=hf * T * (XH * WS) + base * WS + dx,
                                      ap=[[XH * WS, T], [WS, 2], [1, W]])
                        nc.tensor.matmul(psf[32 * ci:32 * ci + 2, :],
                                         wf2_s[T * hf:T * hf + T, hf, tap, :], rhs,
                                         start=(tap == 0), stop=(tap == 8),
                                         tile_position=(T * hf, 32 * ci),
                                         skip_group_check=True)
                bt = sp.tile([128, 2 * W], BF16, tag="bt", name="bt", bufs=2)
                for ci, it in enumerate(its):
                    i0 = 2 * it
                    # rh0 evictions on vector (idle early in phase 3), rh1 on
                    # scalar so vector -- whose SBUF port gpsimd shares -- is
                    # free once the rh0 gathers start
                    if it < 16:
                        nc.vector.tensor_copy(bt[32 * ci:32 * ci + 2, :],
                                              psf[32 * ci:32 * ci + 2, :])
                    else:
                        nc.scalar.copy(bt[32 * ci:32 * ci + 2, :],
                                       psf[32 * ci:32 * ci + 2, :])
                    dst = bass.AP(tensor=flow_rm, offset=i0 * W,
                                  ap=[[64 * W, 2], [1, 2 * W]])
                    nc.sync.dma_start(dst, bt[32 * ci:32 * ci + 2, :])

            # ---- phases 4+5 pipelined by row half rh: flow -> CL2 via DMA
            # transpose XBAR, index math on that half, then its 2 gathers ----
            def cl(tag):
                return bp.tile([128, 128], F32, tag=tag, name=tag)

            clb_fx = bp.tile([128, 128], BF16, tag="clfx")
            clb_fy = bp.tile([128, 128], BF16, tag="clfy")
            ix = cl("ix"); iy = cl("iy"); tmp = cl("tmp")
            x0i = bp.tile([128, 128], I16, tag="x0i")
            y0i = bp.tile([128, 128], I16, tag="y0i")
            x0f = cl("x0f"); y0f = cl("y0f")
            ef = cl("ef")
            eidx = bp.tile([128, 128], I16, tag="eidx")
            gatall = bp.tile([128, 2 * NIDX * DCH], BF16, tag="tcat_gat")

            def rsl(t, rh):
                # both w-halves of one rh: f in {32rh..32rh+32} u {64+32rh..}
                return bass.AP(tensor=t.tensor, offset=32 * rh,
                               ap=[[128, 128], [64, 2], [1, 32]])

            V = nc.vector
            for rh in range(2):
                for ch, dtile in ((0, clb_fx), (1, clb_fy)):
                    for w in range(2):
                        src = bass.AP(tensor=flow_rm,
                                      offset=ch * 64 * W + 32 * rh * W + 128 * w,
                                      ap=[[W, 32], [1, 128]])
                        nc.sync.dma_start(
                            dtile[:, 64 * w + 32 * rh:64 * w + 32 * rh + 32],
                            src, transpose=True)
                V.tensor_scalar_mul(rsl(ix, rh), rsl(clb_fx, rh), 0.5)
                V.tensor_tensor(rsl(ix, rh), rsl(ix, rh), rsl(bx_s, rh), ALU.add)
                V.tensor_scalar_mul(rsl(iy, rh), rsl(clb_fy, rh), 0.5)
                V.tensor_tensor(rsl(iy, rh), rsl(iy, rh), rsl(by_s, rh), ALU.add)
                V.tensor_copy(rsl(x0i, rh), rsl(ix, rh))
                V.tensor_copy(rsl(x0f, rh), rsl(x0i, rh))
                V.tensor_tensor(rsl(tmp, rh), rsl(x0f, rh), rsl(ix, rh), ALU.is_gt)
                V.tensor_tensor(rsl(x0f, rh), rsl(x0f, rh), rsl(tmp, rh),
                                ALU.subtract)
                V.tensor_copy(rsl(y0i, rh), rsl(iy, rh))
                V.tensor_copy(rsl(y0f, rh), rsl(y0i, rh))
                V.tensor_tensor(rsl(tmp, rh), rsl(y0f, rh), rsl(iy, rh), ALU.is_gt)
                V.tensor_tensor(rsl(y0f, rh), rsl(y0f, rh), rsl(tmp, rh),
                                ALU.subtract)
                V.tensor_scalar_mul(rsl(ef, rh), rsl(y0f, rh), float(XS))
                V.tensor_tensor(rsl(ef, rh), rsl(ef, rh), rsl(x0f, rh), ALU.add)
                V.tensor_scalar(rsl(ef, rh), rsl(ef, rh), 0.0,
                                float(LNUM - XS - 2), ALU.max, ALU.min)
                V.tensor_copy(rsl(eidx, rh), rsl(ef, rh))
                for w in range(2):
                    sl = slice(64 * w + 32 * rh, 64 * w + 32 * rh + 32)
                    ds = dsrc if w == 0 else dsrc2
                    off = w * (NIDX * DCH) + rh * 4096
                    nc.gpsimd.ap_gather(
                        gatall[:, off:off + 4096], ds[:],
                        eidx[:, sl],
                        channels=128, num_elems=LNUM, d=DCH, num_idxs=512)

            # ---- weights math (full tensors) ----
            fx = cl("fx"); fy = cl("fy")
            nc.vector.tensor_tensor(fx[:], ix[:], x0f[:], ALU.subtract)
            nc.vector.tensor_tensor(fy[:], iy[:], y0f[:], ALU.subtract)
            vx0 = cl("vx0"); vx1 = cl("vx1"); vy0 = cl("vy0"); vy1 = cl("vy1")
            xp1 = cl("xp1"); yp1 = cl("yp1")
            nc.vector.tensor_scalar_add(xp1[:], x0f[:], 1.0)
            nc.vector.tensor_scalar_add(yp1[:], y0f[:], 1.0)
            for vt, src_f in ((vx0, x0f), (vx1, xp1)):
                nc.vector.tensor_tensor(vt[:], src_f[:], xlo_s[:], ALU.is_ge)
                nc.vector.tensor_tensor(tmp[:], src_f[:], xhi_s[:], ALU.is_le)
                nc.vector.tensor_tensor(vt[:], vt[:], tmp[:], ALU.mult)
            for vt, src_f in ((vy0, y0f), (vy1, yp1)):
                nc.vector.tensor_tensor(vt[:], src_f[:], ylo_s[:], ALU.is_ge)
                nc.vector.tensor_tensor(tmp[:], src_f[:], yhi_s[:], ALU.is_le)
                nc.vector.tensor_tensor(vt[:], vt[:], tmp[:], ALU.mult)
            gx0 = cl("gx0"); gx1 = cl("gx1"); gy0 = cl("gy0"); gy1 = cl("gy1")
            nc.vector.tensor_scalar(tmp[:], fx[:], -1.0, 1.0, ALU.mult, ALU.add)
            nc.vector.tensor_tensor(gx0[:], tmp[:], vx0[:], ALU.mult)
            nc.vector.tensor_tensor(gx1[:], fx[:], vx1[:], ALU.mult)
            nc.vector.tensor_scalar(tmp[:], fy[:], -1.0, 1.0, ALU.mult, ALU.add)
            nc.vector.tensor_tensor(gy0[:], tmp[:], vy0[:], ALU.mult)
            nc.vector.tensor_tensor(gy1[:], fy[:], vy1[:], ALU.mult)
            wsall = bp.tile([128, 4, 128], BF16, tag="wsall")
            nc.vector.tensor_tensor(wsall[:, 0, :], gx0[:], gy0[:], ALU.mult)
            nc.vector.tensor_tensor(wsall[:, 1, :], gx1[:], gy0[:], ALU.mult)
            nc.vector.tensor_tensor(wsall[:, 2, :], gx0[:], gy1[:], ALU.mult)
            nc.vector.tensor_tensor(wsall[:, 3, :], gx1[:], gy1[:], ALU.mult)
            # ---- phase 6: weight planes -> dram -> w_g -> wp01/wp23 ----
            # wp01[p, call*2048 + n*2 + s] = ws_{s}(pixel n), s in {0,1};
            # wp23 likewise for s in {2,3}.  Built j-ordered + pair-interleaved
            # straight out of the emat matmul by reordering its rhs AP, so all
            # copies and all phase-7 reads are (pair-)contiguous.
            nc.sync.dma_start(
                bass.AP(tensor=w_dram, offset=0, ap=[[512, 128], [1, 512]]),
                wsall[:])
            wp = [bp.tile([128, 2 * 2048], BF16, tag=f"wp{i}", name=f"wp{i}")
                  for i in range(2)]
            for spair in range(2):
                w_g = sp.tile([8, 2 * 2048], BF16, tag="wg", name=f"wg{spair}",
                              bufs=1)
                for s4 in range(2):
                    nc.sync.dma_start(
                        w_g[:, s4 * 2048:(s4 + 1) * 2048],
                        bass.AP(tensor=w_dram,
                                offset=(2 * spair + s4) * 128,
                                ap=[[8192, 8], [512, 16], [1, 128]]))
                for call in range(2):
                    for rh in range(2):
                        for jjh in range(2):
                            pw = pp.tile([128, 512], F32, tag="psf", name="pw",
                                         bufs=3)
                            rhs = bass.AP(
                                tensor=w_g.tensor,
                                offset=64 * call + 32 * rh + 16 * jjh,
                                ap=[[2 * 2048, 8], [1, 16], [128, 16], [2048, 2]])
                            nc.tensor.matmul(pw[:], emat_s[:], rhs,
                                             start=True, stop=True)
                            dst = wp[spair][:, call * 2048 + rh * 1024
                                            + jjh * 512:][:, 0:512]
                            nc.scalar.copy(dst, pw[:])

            P01 = bp.tile([128, 2 * NIDX], BF16, tag="P01")
            P23 = bp.tile([128, 2 * NIDX], BF16, tag="P23")
            bb_s = [bp.tile([128, NIDX], F32, tag=f"bb{i}", name=f"bb{i}")
                    for i in range(2)]
            for call in range(2):
                for slot in range(2):
                    def g_pair(sbase):
                        return bass.AP(tensor=gatall.tensor,
                                       offset=call * NIDX * DCH + 4 * slot + sbase,
                                       ap=[[2 * NIDX * DCH, 128], [DCH, NIDX],
                                           [1, 2]])
                    wslice0 = wp[0][:, call * 2048:(call + 1) * 2048]
                    wslice1 = wp[1][:, call * 2048:(call + 1) * 2048]
                    bb = bb_s[slot]
                    nc.vector.tensor_tensor(P01[:], g_pair(0), wslice0, ALU.mult)
                    nc.vector.tensor_tensor(P23[:], g_pair(2), wslice1, ALU.mult)
                    nc.vector.tensor_tensor(P01[:], P01[:], P23[:], ALU.add)
                    pv0 = bass.AP(tensor=P01.tensor, offset=0,
                                  ap=[[2 * NIDX, 128], [2, NIDX]])
                    pv1 = bass.AP(tensor=P01.tensor, offset=1,
                                  ap=[[2 * NIDX, 128], [2, NIDX]])
                    nc.vector.tensor_tensor(bb[:], pv0, pv1, ALU.add)
                    (nc.scalar if (call + slot) % 2 == 0 else nc.sync).dma_start(
                        out_d[call, slot], bb[:])
    nc.finalize()
    return nc


# ======================= host-side prep =======================

def _feat_slab(feat_b, h):
    """feat_b (256, 128, 256) f32 -> (2, 128, 68, 256) bf16 slab for half h."""
    r0 = 64 * h - 2
    slab = np.zeros((CIN, SLAB_R, W), np.float32)
    lo, hi = max(r0, 0), min(r0 + SLAB_R, H)
    slab[:, lo - r0:hi - r0, :] = feat_b[:, lo:hi, :]
    return np.ascontiguousarray(
        slab.reshape(2, 128, SLAB_R, W).astype(BF))


def _host_constants(q, h):
    R0 = 64 * h
    # CL2 layout: p = 16G + m, f = 64w + r; pixel (row R0+r, col 128w+16G+m)
    p = np.arange(128)[:, None]
    f = np.arange(128)[None, :]
    G = p // 16
    m = p % 16
    r = f % 64
    w = f // 64
    col = 128 * w + 16 * G + m
    row = R0 + r
    ix_base = col + col / (W - 1.0) - 0.5
    iy_base = row + row / (H - 1.0) - 0.5
    colbase = 128 * w + 16 * G - COLB
    rowbase = R0 - ROWB
    bx = np.broadcast_to(ix_base - colbase, (128, 128)).astype(np.float32).copy()
    by = np.broadcast_to(iy_base - rowbase, (128, 128)).astype(np.float32).copy()
    xlo = np.broadcast_to(0.0 - colbase, (128, 128)).astype(np.float32).copy()
    xhi = np.broadcast_to((W - 1.0) - colbase, (128, 128)).astype(np.float32).copy()
    ylo = np.full((128, 128), 0.0 - rowbase, np.float32)
    yhi = np.full((128, 128), (H - 1.0) - rowbase, np.float32)
    return bx, by, xlo, xhi, ylo, yhi


def _dsrc_build(pred_imgs, h):
    """pred_imgs: (19, 128, 256) f32. Returns (2, 128, LNUM*8) f32 gather
    source; call = col-half w, slab = rows [R0-6, R0+70) x 26-col band."""
    R0 = 64 * h
    padded = np.zeros((CCLS, H + 16, W + 16), np.float32)
    padded[:, 8:8 + H, 8:8 + W] = pred_imgs
    out = np.zeros((2, 128, LNUM, DCH), np.float32)
    rowbase = R0 - ROWB
    for call in range(2):
        for G in range(8):
            colbase = 128 * call + 16 * G - COLB
            for l in range(16):
                for slot in range(2):
                    img = l + 16 * slot
                    if img >= CCLS:
                        img = l
                    for j2 in range(2):
                        for j1 in range(2):
                            win = padded[img,
                                         8 + rowbase + j2: 8 + rowbase + j2 + YS,
                                         8 + colbase + j1: 8 + colbase + j1 + XS]
                            out[call, 16 * G + l, :, 4 * slot + 2 * j2 + j1] = \
                                win.reshape(-1)
    return out.reshape(2, 128, LNUM * DCH)


def make_inputs(core, t1_feature, t2_feature, t2_pred, w_down1, w_down2,
                w_flow1, bn_gamma, bn_beta, bn_mean, bn_var, w_flow2):
    q, h = core // 2, core % 2
    f1 = _feat_slab(t1_feature[q], h)
    f2 = _feat_slab(t2_feature[q], h)
    wd = np.stack([
        np.stack([w_down1[:, 128 * k:128 * (k + 1), 0, 0].T for k in range(2)]),
        np.stack([w_down2[:, 128 * k:128 * (k + 1), 0, 0].T for k in range(2)]),
    ]).transpose(2, 0, 1, 3).astype(BF).copy()        # (128,2,2,64)
    wf1 = np.stack([w_flow1[:, :, t // 3, t % 3].T for t in range(9)],
                   axis=1).astype(BF).copy()          # (128,9,64)
    wf2h = np.stack([w_flow2[:, :, t // 3, t % 3].T for t in range(9)],
                    axis=1).astype(BF)                # (64,9,2)
    z = np.zeros_like(wf2h)
    wf2 = np.stack([np.concatenate([wf2h, z], axis=0),
                    np.concatenate([z, wf2h], axis=0)],
                   axis=1).copy()                     # (128,2,9,2)
    scale = bn_gamma / np.sqrt(bn_var + 1e-5)
    bias = bn_beta - bn_mean * scale
    bn1 = np.stack([scale, bias], axis=1).reshape(T, 2, 1).astype(np.float32)
    bn = np.concatenate([bn1, bn1], axis=0)           # (128,2,1)
    mask = np.ones((128, 2, 1), np.float32)
    if h == 0:
        mask[0:T, 0] = 0.0   # x row 0 (half A) = image row -1
    else:
        mask[T:128, 1] = 0.0  # x half-B row 35 = x row 65 = image row 128
    bx, by, xlo, xhi, ylo, yhi = _host_constants(q, h)
    imgs = img_list(q)
    pred_imgs = np.stack([t2_pred[n, ch] for (n, ch) in imgs])
    dsrc = _dsrc_build(pred_imgs, h)
    emat = np.zeros((8, 128), BF)
    for Gi in range(8):
        emat[Gi, 16 * Gi:16 * (Gi + 1)] = 1.0
    return {
        "f1": f1, "f2": f2, "wd": wd, "wf1": wf1, "wf2": wf2,
        "bn": bn, "mask": mask, "bx": bx, "by": by, "ylo": ylo, "yhi": yhi,
        "xlo": xlo, "xhi": xhi, "emat": emat, "dsrc": dsrc.astype(BF),
    }


def unshard_out(raw):
    """raw (2, 2, 128, 1024) f32 -> (19-ish, 64, 256) per-image rows.

    raw[call, slot, 16G+l, 512rh+16jj+m] = out[img(l,slot), 32rh+jj,
    128call+16G+m].
    """
    o = raw.reshape(2, 2, 8, 16, 2, 32, 16)  # call, slot, G, l, rh, jj, m
    full = o.transpose(1, 3, 4, 5, 0, 2, 6).reshape(2, 16, 64, 256)
    return full  # [slot, l, r, col]


def kernel(**inputs):
    from concourse.bass_utils import run_bass_kernel_spmd
    if "nc" not in _NC_CACHE:
        _NC_CACHE["nc"] = build_nc()
    nc = _NC_CACHE["nc"]
    in_maps = [make_inputs(c, **inputs) for c in range(8)]
    res = run_bass_kernel_spmd(nc, in_maps, list(range(8)))
    out = np.zeros((NB, CCLS, H, W), np.float32)
    for c in range(8):
        q, h = c // 2, c % 2
        full = unshard_out(res.results[c]["out"])
        for i, (n, ch) in enumerate(img_list(q)):
            out[n, ch, 64 * h:64 * (h + 1), :] = full[i // 16, i % 16]
    return out


_NC_CACHE = {}


# revision 41
# speedup vs baseline: 1.0292x; 1.0292x over previous
"""Trainium2 Bass kernel for nn_AlignModule_full (8 NeuronCores, data-parallel).

Reference computation: two 1x1 convs -> concat -> 3x3 conv + BN + ReLU ->
3x3 conv -> flow -> bilinear grid_sample warp of t2_pred, where output
channel (n, ch) is warped with flow[(3n+ch) % 4] (torch flow.repeat
semantics faithfully ported by the reference).

Sharding: core c = (q, h), q = c//2 flow batch, h = c%2 row half.
Each core computes flow(q, rows 64h..64h+64) from batch-q features, then
warps the 19 (n, ch) images with (3n+ch)%4 == q for its row half, using
only its own flow. Zero cross-core communication.

v2 layout notes:
- feature loads in 4-row chunks (big DMA descriptors) on sync+vector queues
- phase 3 (3x3 conv 64->2) runs 3 col-tiled x 2 row-tiled concurrent
  matmuls via tile_position (M=2 only needs a 32-wide PE strip)
- flow goes to DRAM row-major bf16 (2KB descriptors), comes back through
  the DMA transpose XBAR into CL2 layout: partition p = 16G+m,
  free f = 64w+r, pixel (row 64h+r, col 128w+16G+m)
- output is dumped as raw [call, slot, 128, 1024] tiles; host unshards
"""
import sys

for _p in ('/opt/trn_rl_repo',):
    if _p not in sys.path:
        sys.path.append(_p)

import numpy as np
import ml_dtypes

import concourse.bass as bass
import concourse.bacc as bacc
import concourse.mybir as mybir
import concourse.tile as tile

F32 = mybir.dt.float32
BF16 = mybir.dt.bfloat16
I16 = mybir.dt.int16
AF = mybir.ActivationFunctionType
ALU = mybir.AluOpType

H, W, CIN, T, CCLS, NB = 128, 256, 256, 64, 19, 4
SLAB_R = 68          # feature slab rows
WS = 258             # padded width for t/x buffers
XR = 66              # x rows total
XH = 36              # x rows per partition-half (A: 0..36, B: 30..66)
YS, XS = 76, 26      # gather slab rows/cols per (group, call=col-half)
LNUM = YS * XS       # base positions per partition
DCH = 8              # interleave chunk: 2 slots x (2x2 patch)
NIDX = 1024          # gather indices per group per call
ROWB = 6             # slab row margin before first output row of the call
COLB = 5             # slab col margin before group col block

BF = ml_dtypes.bfloat16


def img_list(q):
    return [(n, ch) for n in range(NB) for ch in range(CCLS)
            if (3 * n + ch) % 4 == q]


def build_nc():
    nc = bacc.Bacc(None, target_bir_lowering=False, debug=False)
    P = nc.declare_dram_parameter
    f1_d = P("f1", [2, 128, SLAB_R, W], BF16, isOutput=False)
    f2_d = P("f2", [2, 128, SLAB_R, W], BF16, isOutput=False)
    wd_d = P("wd", [128, 2, 2, T], BF16, isOutput=False)
    wf1_d = P("wf1", [128, 9, T], BF16, isOutput=False)
    wf2_d = P("wf2", [128, 2, 9, 2], BF16, isOutput=False)
    bn_d = P("bn", [128, 2, 1], F32, isOutput=False)
    mask_d = P("mask", [128, 2, 1], F32, isOutput=False)
    bx_d = P("bx", [128, 128], F32, isOutput=False)
    by_d = P("by", [128, 128], F32, isOutput=False)
    ylo_d = P("ylo", [128, 128], F32, isOutput=False)
    yhi_d = P("yhi", [128, 128], F32, isOutput=False)
    xlo_d = P("xlo", [128, 128], F32, isOutput=False)
    xhi_d = P("xhi", [128, 128], F32, isOutput=False)
    emat_d = P("emat", [8, 128], BF16, isOutput=False)
    dsrc_d = P("dsrc", [2, 128, LNUM * DCH], BF16, isOutput=False)
    out_d = P("out", [2, 2, 128, NIDX], F32, isOutput=True)

    flow_rm = nc.dram_tensor("flow_rm", [2, 64, W], BF16)   # row-major bf16
    w_dram = nc.dram_tensor("w_dram", [4 * 128 * 128], BF16)

    NCH = 9  # feature chunks of 8 rows (last chunk 4)

    with tile.TileContext(nc) as tc:
        with (
            tc.tile_pool(name="stream", bufs=3) as sp,
            tc.tile_pool(name="big", bufs=1) as bp,
            tc.tile_pool(name="psA", bufs=2, space="PSUM") as pp,
        ):
            # ---- constants ----
            wd_s = bp.tile([128, 2, 2, T], BF16, tag="wd")
            wf1_s = bp.tile([128, 9, T], BF16, tag="wf1")
            wf2_s = bp.tile([128, 2, 9, 2], BF16, tag="wf2")
            bn_s = bp.tile([128, 2, 1], F32, tag="bn")
            mask_s = bp.tile([128, 2, 1], F32, tag="mask")
            xlo_s = bp.tile([128, 128], F32, tag="xlo")
            xhi_s = bp.tile([128, 128], F32, tag="xhi")
            emat_s = bp.tile([8, 128], BF16, tag="emat")
            bx_s = bp.tile([128, 128], F32, tag="bx")
            by_s = bp.tile([128, 128], F32, tag="by")
            ylo_s = bp.tile([128, 128], F32, tag="ylo")
            yhi_s = bp.tile([128, 128], F32, tag="yhi")
            nc.sync.dma_start(wd_s[:], wd_d[:])

            # ---- big shared tiles ----
            t_cat = bp.tile([128, SLAB_R * WS], BF16, tag="tcat_gat")
            dsrc = bp.tile([128, LNUM * DCH], BF16, tag="dsrc")
            dsrc2 = bp.tile([128, LNUM * DCH], BF16, tag="dsrc2")
            x_sb = bp.tile([128, XH * WS], BF16, tag="x_w4")

            t3 = t_cat[:].rearrange("p (r c) -> p r c", r=SLAB_R, c=WS)
            nc.vector.memset(t3[:, :, 0:1], 0.0)
            nc.vector.memset(t3[:, :, 257:258], 0.0)

            # ---- feature chunk loads: 4 rows per chunk, 2 DMAs each ----
            fch = {}

            def load_chunk(k):
                c1 = sp.tile([128, 2, 8, W], BF16, tag="fc1", name=f"fc1_{k}",
                             bufs=2)
                c2 = sp.tile([128, 2, 8, W], BF16, tag="fc2", name=f"fc2_{k}",
                             bufs=2)
                r0 = 8 * k
                nr = min(8, SLAB_R - r0)
                qa, qb = (nc.sync, nc.scalar) if k % 2 == 0 else (nc.scalar, nc.sync)
                qa.dma_start(c1[:, 0, 0:nr], f1_d[0, :, r0:r0 + nr, :])
                qb.dma_start(c1[:, 1, 0:nr], f1_d[1, :, r0:r0 + nr, :])
                qa.dma_start(c2[:, 0, 0:nr], f2_d[0, :, r0:r0 + nr, :])
                qb.dma_start(c2[:, 1, 0:nr], f2_d[1, :, r0:r0 + nr, :])
                fch[k] = (c1, c2)

            for k in range(2):
                load_chunk(k)
            for t_, d_ in ((wf1_s, wf1_d), (bn_s, bn_d)):
                nc.sync.dma_start(t_[:], d_[:])
            for t_, d_ in ((wf2_s, wf2_d),
                           (mask_s, mask_d), (xlo_s, xlo_d),
                           (xhi_s, xhi_d),
                           (emat_s, emat_d), (bx_s, bx_d), (by_s, by_d),
                           (ylo_s, ylo_d), (yhi_s, yhi_d)):
                nc.scalar.dma_start(t_[:], d_[:])

            # warm the gpsimd ext-isa gather lib early so the real gathers
            # (and nothing else on gpsimd) don't pay the IRAM load later
            widx = bp.tile([128, 4], I16, tag="widx")
            wout = bp.tile([128, 128], BF16, tag="wout")
            wsrc = bp.tile([128, 256], BF16, tag="wsrc")
            nc.vector.memset(widx[:], 0)
            nc.vector.memset(wsrc[:], 0.0)
            nc.gpsimd.ap_gather(
                wout[:],
                wsrc[:].rearrange("p (n d) -> p n d", d=8),
                widx[:, 0:1],
                channels=128, num_elems=32, d=8, num_idxs=16)

            # ---- phases 1+2 interleaved: 1x1 convs feed 3x3 conv ----
            def p1_tile(it):
                r0 = 2 * it
                k, lr = it // 4, 2 * (it % 4)
                if lr == 0 and (k + 2) < NCH and (k + 2) not in fch:
                    load_chunk(k + 2)
                c1, c2 = fch[k]
                ps = pp.tile([128, 2 * W], F32, tag="pst", name="pst")
                for ck in range(2):
                    nc.tensor.matmul(ps[0:T, :], wd_s[:, 0, ck, :],
                                     c1[:, ck, lr:lr + 2, :],
                                     start=(ck == 0), stop=(ck == 1))
                for ck in range(2):
                    nc.tensor.matmul(ps[T:128, :], wd_s[:, 1, ck, :],
                                     c2[:, ck, lr:lr + 2, :],
                                     start=(ck == 0), stop=(ck == 1))
                dst = bass.AP(tensor=t_cat.tensor, offset=r0 * WS + 1,
                              ap=[[SLAB_R * WS, 128], [WS, 2], [1, W]])
                nc.vector.tensor_copy(dst, ps[:].rearrange("p (r c) -> p r c",
                                                           r=2, c=W))

            x3 = x_sb[:].rearrange("p (r c) -> p r c", r=XH, c=WS)
            nc.vector.memset(x3[:, :, 0:1], 0.0)
            nc.vector.memset(x3[:, :, 257:258], 0.0)

            def p2_iter(it):
                jA = 2 * it
                jB = 30 + 2 * it
                ps = pp.tile([128, 2 * W], F32, tag="psx", name="psx")
                for tap in range(9):
                    dy, dx = tap // 3, tap % 3
                    rhsA = bass.AP(tensor=t_cat.tensor,
                                   offset=(jA + dy) * WS + dx,
                                   ap=[[SLAB_R * WS, 128], [WS, 2], [1, W]])
                    rhsB = bass.AP(tensor=t_cat.tensor,
                                   offset=(jB + dy) * WS + dx,
                                   ap=[[SLAB_R * WS, 128], [WS, 2], [1, W]])
                    nc.tensor.matmul(ps[0:T, :], wf1_s[:, tap, :], rhsA,
                                     start=(tap == 0), stop=(tap == 8),
                                     tile_position=(0, 0),
                                     skip_group_check=True)
                    nc.tensor.matmul(ps[T:128, :], wf1_s[:, tap, :], rhsB,
                                     start=(tap == 0), stop=(tap == 8),
                                     tile_position=(0, 64),
                                     skip_group_check=True)
                dstA = bass.AP(tensor=x_sb.tensor, offset=jA * WS + 1,
                               ap=[[XH * WS, T], [WS, 2], [1, W]])
                dstB = bass.AP(tensor=x_sb.tensor,
                               offset=T * (XH * WS) + jA * WS + 1,
                               ap=[[XH * WS, T], [WS, 2], [1, W]])
                nc.scalar.activation(dstA,
                                     ps[0:T].rearrange("p (r c) -> p r c", r=2, c=W),
                                     AF.Relu, bias=bn_s[0:T, 1], scale=bn_s[0:T, 0])
                nc.scalar.activation(dstB,
                                     ps[T:128].rearrange("p (r c) -> p r c", r=2, c=W),
                                     AF.Relu, bias=bn_s[T:128, 1], scale=bn_s[T:128, 0])

            for it in range(34):
                p1_tile(it)
                if it >= 16:
                    p2_iter(it - 16)
            nc.vector.tensor_scalar_mul(x3[0:T, 0, :], x3[0:T, 0, :], mask_s[0:T, 0])
            nc.vector.tensor_scalar_mul(x3[T:128, 35, :], x3[T:128, 35, :],
                                        mask_s[T:128, 1])

            # dsrc loads on sync/scalar (keep gpsimd gather-only so its
            # ext-isa lib stays resident).  Gated so the scheduler doesn't
            # hoist these 2MB transfers into the feature-load window.
            with tc.tile_wait_until(0.14):
                nc.scalar.dma_start(dsrc[:, 0:LNUM * DCH // 2],
                                    dsrc_d[0, :, 0:LNUM * DCH // 2])
                nc.scalar.dma_start(dsrc[:, LNUM * DCH // 2:],
                                    dsrc_d[0, :, LNUM * DCH // 2:])
            with tc.tile_wait_until(0.17):
                nc.scalar.dma_start(dsrc2[:, 0:LNUM * DCH // 2],
                                    dsrc_d[1, :, 0:LNUM * DCH // 2])
                nc.scalar.dma_start(dsrc2[:, LNUM * DCH // 2:],
                                    dsrc_d[1, :, LNUM * DCH // 2:])

            # ---- phase 3: 3x3 conv 64->2, groups of 3 col-tiled chunks ----
            # chunk it -> flow rows (2it, 2it+1); it 0..15 from x half A
            # (row tile 0), it 16..31 from half B (row tile 64).
            for g in range(11):
                its = [i for i in (3 * g, 3 * g + 1, 3 * g + 2) if i < 32]
                psf = pp.tile([128, 2 * W], F32, tag="psf", name="psf", bufs=3)
                for tap in range(9):
                    dy, dx = tap // 3, tap % 3
                    for ci, it in enumerate(its):
                        i0 = 2 * it
                        hf = it // 16
                        base = i0 + dy - 30 * hf
                        rhs = bass.AP(tensor=x_sb.tensor,
                                      offset=hf * T * (XH * WS) + base * WS + dx,
                                      ap=[[XH * WS, T], [WS, 2], [1, W]])
                        nc.tensor.matmul(psf[32 * ci:32 * ci + 2, :],
                                         wf2_s[T * hf:T * hf + T, hf, tap, :], rhs,
                                         start=(tap == 0), stop=(tap == 8),
                                         tile_position=(T * hf, 32 * ci),
                                         skip_group_check=True)
                bt = sp.tile([128, 2 * W], BF16, tag="bt", name="bt", bufs=4)
                for ci, it in enumerate(its):
                    i0 = 2 * it
                    # rh0 evictions on vector (idle early in phase 3), rh1 on
                    # scalar so vector -- whose SBUF port gpsimd shares -- is
                    # free once the rh0 gathers start
                    if it < 16:
                        nc.vector.tensor_copy(bt[32 * ci:32 * ci + 2, :],
                                              psf[32 * ci:32 * ci + 2, :])
                    else:
                        nc.scalar.copy(bt[32 * ci:32 * ci + 2, :],
                                       psf[32 * ci:32 * ci + 2, :])
                    dst = bass.AP(tensor=flow_rm, offset=i0 * W,
                                  ap=[[64 * W, 2], [1, 2 * W]])
                    nc.sync.dma_start(dst, bt[32 * ci:32 * ci + 2, :])

            # ---- phases 4+5 pipelined by row half rh: flow -> CL2 via DMA
            # transpose XBAR, index math on that half, then its 2 gathers ----
            def cl(tag):
                return bp.tile([128, 128], F32, tag=tag, name=tag)

            clb_fx = bp.tile([128, 128], BF16, tag="clfx")
            clb_fy = bp.tile([128, 128], BF16, tag="clfy")
            ix = cl("ix"); iy = cl("iy"); tmp = cl("tmp")
            x0i = bp.tile([128, 128], I16, tag="x0i")
            y0i = bp.tile([128, 128], I16, tag="y0i")
            x0f = cl("x0f"); y0f = cl("y0f")
            ef = cl("ef")
            eidx = bp.tile([128, 128], I16, tag="eidx")
            gatall = bp.tile([128, 2 * NIDX * DCH], BF16, tag="tcat_gat")

            def rsl(t, rh):
                # both w-halves of one rh: f in {32rh..32rh+32} u {64+32rh..}
                return bass.AP(tensor=t.tensor, offset=32 * rh,
                               ap=[[128, 128], [64, 2], [1, 32]])

            V = nc.vector
            for rh in range(2):
                for ch, dtile in ((0, clb_fx), (1, clb_fy)):
                    for w in range(2):
                        src = bass.AP(tensor=flow_rm,
                                      offset=ch * 64 * W + 32 * rh * W + 128 * w,
                                      ap=[[W, 32], [1, 128]])
                        nc.sync.dma_start(
                            dtile[:, 64 * w + 32 * rh:64 * w + 32 * rh + 32],
                            src, transpose=True)
                V.tensor_scalar_mul(rsl(ix, rh), rsl(clb_fx, rh), 0.5)
                V.tensor_tensor(rsl(ix, rh), rsl(ix, rh), rsl(bx_s, rh), ALU.add)
                V.tensor_scalar_mul(rsl(iy, rh), rsl(clb_fy, rh), 0.5)
                V.tensor_tensor(rsl(iy, rh), rsl(iy, rh), rsl(by_s, rh), ALU.add)
                V.tensor_copy(rsl(x0i, rh), rsl(ix, rh))
                V.tensor_copy(rsl(x0f, rh), rsl(x0i, rh))
                V.tensor_tensor(rsl(tmp, rh), rsl(x0f, rh), rsl(ix, rh), ALU.is_gt)
                V.tensor_tensor(rsl(x0f, rh), rsl(x0f, rh), rsl(tmp, rh),
                                ALU.subtract)
                V.tensor_copy(rsl(y0i, rh), rsl(iy, rh))
                V.tensor_copy(rsl(y0f, rh), rsl(y0i, rh))
                V.tensor_tensor(rsl(tmp, rh), rsl(y0f, rh), rsl(iy, rh), ALU.is_gt)
                V.tensor_tensor(rsl(y0f, rh), rsl(y0f, rh), rsl(tmp, rh),
                                ALU.subtract)
                V.tensor_scalar_mul(rsl(ef, rh), rsl(y0f, rh), float(XS))
                V.tensor_tensor(rsl(ef, rh), rsl(ef, rh), rsl(x0f, rh), ALU.add)
                V.tensor_scalar(rsl(ef, rh), rsl(ef, rh), 0.0,
                                float(LNUM - XS - 2), ALU.max, ALU.min)
                V.tensor_copy(rsl(eidx, rh), rsl(ef, rh))
                for w in range(2):
                    sl = slice(64 * w + 32 * rh, 64 * w + 32 * rh + 32)
                    ds = dsrc if w == 0 else dsrc2
                    off = w * (NIDX * DCH) + rh * 4096
                    nc.gpsimd.ap_gather(
                        gatall[:, off:off + 4096], ds[:],
                        eidx[:, sl],
                        channels=128, num_elems=LNUM, d=DCH, num_idxs=512)

            # ---- weights math (full tensors) ----
            fx = cl("fx"); fy = cl("fy")
            nc.vector.tensor_tensor(fx[:], ix[:], x0f[:], ALU.subtract)
            nc.vector.tensor_tensor(fy[:], iy[:], y0f[:], ALU.subtract)
            vx0 = cl("vx0"); vx1 = cl("vx1"); vy0 = cl("vy0"); vy1 = cl("vy1")
            xp1 = cl("xp1"); yp1 = cl("yp1")
            nc.vector.tensor_scalar_add(xp1[:], x0f[:], 1.0)
            nc.vector.tensor_scalar_add(yp1[:], y0f[:], 1.0)
            for vt, src_f in ((vx0, x0f), (vx1, xp1)):
                nc.vector.tensor_tensor(vt[:], src_f[:], xlo_s[:], ALU.is_ge)
                nc.vector.tensor_tensor(tmp[:], src_f[:], xhi_s[:], ALU.is_le)
                nc.vector.tensor_tensor(vt[:], vt[:], tmp[:], ALU.mult)
            for vt, src_f in ((vy0, y0f), (vy1, yp1)):
                nc.vector.tensor_tensor(vt[:], src_f[:], ylo_s[:], ALU.is_ge)
                nc.vector.tensor_tensor(tmp[:], src_f[:], yhi_s[:], ALU.is_le)
                nc.vector.tensor_tensor(vt[:], vt[:], tmp[:], ALU.mult)
            gx0 = cl("gx0"); gx1 = cl("gx1"); gy0 = cl("gy0"); gy1 = cl("gy1")
            nc.vector.tensor_scalar(tmp[:], fx[:], -1.0, 1.0, ALU.mult, ALU.add)
            nc.vector.tensor_tensor(gx0[:], tmp[:], vx0[:], ALU.mult)
            nc.vector.tensor_tensor(gx1[:], fx[:], vx1[:], ALU.mult)
            nc.vector.tensor_scalar(tmp[:], fy[:], -1.0, 1.0, ALU.mult, ALU.add)
            nc.vector.tensor_tensor(gy0[:], tmp[:], vy0[:], ALU.mult)
            nc.vector.tensor_tensor(gy1[:], fy[:], vy1[:], ALU.mult)
            wsall = bp.tile([128, 4, 128], BF16, tag="wsall")
            nc.vector.tensor_tensor(wsall[:, 0, :], gx0[:], gy0[:], ALU.mult)
            nc.vector.tensor_tensor(wsall[:, 1, :], gx1[:], gy0[:], ALU.mult)
            nc.vector.tensor_tensor(wsall[:, 2, :], gx0[:], gy1[:], ALU.mult)
            nc.vector.tensor_tensor(wsall[:, 3, :], gx1[:], gy1[:], ALU.mult)
            # ---- phase 6: weight planes -> dram -> w_g -> wp01/wp23 ----
            # wp01[p, call*2048 + n*2 + s] = ws_{s}(pixel n), s in {0,1};
            # wp23 likewise for s in {2,3}.  Built j-ordered + pair-interleaved
            # straight out of the emat matmul by reordering its rhs AP, so all
            # copies and all phase-7 reads are (pair-)contiguous.
            nc.sync.dma_start(
                bass.AP(tensor=w_dram, offset=0, ap=[[512, 128], [1, 512]]),
                wsall[:])
            wp = [bp.tile([128, 2 * 2048], BF16, tag=f"wp{i}", name=f"wp{i}")
                  for i in range(2)]
            for spair in range(2):
                w_g = sp.tile([8, 2 * 2048], BF16, tag="wg", name=f"wg{spair}",
                              bufs=1)
                for s4 in range(2):
                    nc.sync.dma_start(
                        w_g[:, s4 * 2048:(s4 + 1) * 2048],
                        bass.AP(tensor=w_dram,
                                offset=(2 * spair + s4) * 128,
                                ap=[[8192, 8], [512, 16], [1, 128]]))
                for call in range(2):
                    for rh in range(2):
                        for jjh in range(2):
                            pw = pp.tile([128, 512], F32, tag="psf", name="pw",
                                         bufs=3)
                            rhs = bass.AP(
                                tensor=w_g.tensor,
                                offset=64 * call + 32 * rh + 16 * jjh,
                                ap=[[2 * 2048, 8], [1, 16], [128, 16], [2048, 2]])
                            nc.tensor.matmul(pw[:], emat_s[:], rhs,
                                             start=True, stop=True)
                            dst = wp[spair][:, call * 2048 + rh * 1024
                                            + jjh * 512:][:, 0:512]
                            nc.scalar.copy(dst, pw[:])

            P01 = bp.tile([128, 2 * NIDX], BF16, tag="P01")
            P23 = bp.tile([128, 2 * NIDX], BF16, tag="P23")
            bb_s = [bp.tile([128, NIDX], F32, tag=f"bb{i}", name=f"bb{i}")
                    for i in range(2)]
            for call in range(2):
                for slot in range(2):
                    def g_pair(sbase):
                        return bass.AP(tensor=gatall.tensor,
                                       offset=call * NIDX * DCH + 4 * slot + sbase,
                                       ap=[[2 * NIDX * DCH, 128], [DCH, NIDX],
                                           [1, 2]])
                    wslice0 = wp[0][:, call * 2048:(call + 1) * 2048]
                    wslice1 = wp[1][:, call * 2048:(call + 1) * 2048]
                    bb = bb_s[slot]
                    nc.vector.tensor_tensor(P01[:], g_pair(0), wslice0, ALU.mult)
                    nc.vector.tensor_tensor(P23[:], g_pair(2), wslice1, ALU.mult)
                    nc.vector.tensor_tensor(P01[:], P01[:], P23[:], ALU.add)
                    pv0 = bass.AP(tensor=P01.tensor, offset=0,
                                  ap=[[2 * NIDX, 128], [2, NIDX]])
                    pv1 = bass.AP(tensor=P01.tensor, offset=1,
                                  ap=[[2 * NIDX, 128], [2, NIDX]])
                    nc.vector.tensor_tensor(bb[:], pv0, pv1, ALU.add)
                    (nc.scalar if (call + slot) % 2 == 0 else nc.sync).dma_start(
                        out_d[call, slot], bb[:])
    nc.finalize()
    return nc


# ======================= host-side prep =======================

def _feat_slab(feat_b, h):
    """feat_b (256, 128, 256) f32 -> (2, 128, 68, 256) bf16 slab for half h."""
    r0 = 64 * h - 2
    slab = np.zeros((CIN, SLAB_R, W), np.float32)
    lo, hi = max(r0, 0), min(r0 + SLAB_R, H)
    slab[:, lo - r0:hi - r0, :] = feat_b[:, lo:hi, :]
    return np.ascontiguousarray(
        slab.reshape(2, 128, SLAB_R, W).astype(BF))


def _host_constants(q, h):
    R0 = 64 * h
    # CL2 layout: p = 16G + m, f = 64w + r; pixel (row R0+r, col 128w+16G+m)
    p = np.arange(128)[:, None]
    f = np.arange(128)[None, :]
    G = p // 16
    m = p % 16
    r = f % 64
    w = f // 64
    col = 128 * w + 16 * G + m
    row = R0 + r
    ix_base = col + col / (W - 1.0) - 0.5
    iy_base = row + row / (H - 1.0) - 0.5
    colbase = 128 * w + 16 * G - COLB
    rowbase = R0 - ROWB
    bx = np.broadcast_to(ix_base - colbase, (128, 128)).astype(np.float32).copy()
    by = np.broadcast_to(iy_base - rowbase, (128, 128)).astype(np.float32).copy()
    xlo = np.broadcast_to(0.0 - colbase, (128, 128)).astype(np.float32).copy()
    xhi = np.broadcast_to((W - 1.0) - colbase, (128, 128)).astype(np.float32).copy()
    ylo = np.full((128, 128), 0.0 - rowbase, np.float32)
    yhi = np.full((128, 128), (H - 1.0) - rowbase, np.float32)
    return bx, by, xlo, xhi, ylo, yhi


def _dsrc_build(pred_imgs, h):
    """pred_imgs: (19, 128, 256) f32. Returns (2, 128, LNUM*8) f32 gather
    source; call = col-half w, slab = rows [R0-6, R0+70) x 26-col band."""
    R0 = 64 * h
    padded = np.zeros((CCLS, H + 16, W + 16), np.float32)
    padded[:, 8:8 + H, 8:8 + W] = pred_imgs
    out = np.zeros((2, 128, LNUM, DCH), np.float32)
    rowbase = R0 - ROWB
    for call in range(2):
        for G in range(8):
            colbase = 128 * call + 16 * G - COLB
            for l in range(16):
                for slot in range(2):
                    img = l + 16 * slot
                    if img >= CCLS:
                        img = l
                    for j2 in range(2):
                        for j1 in range(2):
                            win = padded[img,
                                         8 + rowbase + j2: 8 + rowbase + j2 + YS,
                                         8 + colbase + j1: 8 + colbase + j1 + XS]
                            out[call, 16 * G + l, :, 4 * slot + 2 * j2 + j1] = \
                                win.reshape(-1)
    return out.reshape(2, 128, LNUM * DCH)


def make_inputs(core, t1_feature, t2_feature, t2_pred, w_down1, w_down2,
                w_flow1, bn_gamma, bn_beta, bn_mean, bn_var, w_flow2):
    q, h = core // 2, core % 2
    f1 = _feat_slab(t1_feature[q], h)
    f2 = _feat_slab(t2_feature[q], h)
    wd = np.stack([
        np.stack([w_down1[:, 128 * k:128 * (k + 1), 0, 0].T for k in range(2)]),
        np.stack([w_down2[:, 128 * k:128 * (k + 1), 0, 0].T for k in range(2)]),
    ]).transpose(2, 0, 1, 3).astype(BF).copy()        # (128,2,2,64)
    wf1 = np.stack([w_flow1[:, :, t // 3, t % 3].T for t in range(9)],
                   axis=1).astype(BF).copy()          # (128,9,64)
    wf2h = np.stack([w_flow2[:, :, t // 3, t % 3].T for t in range(9)],
                    axis=1).astype(BF)                # (64,9,2)
    z = np.zeros_like(wf2h)
    wf2 = np.stack([np.concatenate([wf2h, z], axis=0),
                    np.concatenate([z, wf2h], axis=0)],
                   axis=1).copy()                     # (128,2,9,2)
    scale = bn_gamma / np.sqrt(bn_var + 1e-5)
    bias = bn_beta - bn_mean * scale
    bn1 = np.stack([scale, bias], axis=1).reshape(T, 2, 1).astype(np.float32)
    bn = np.concatenate([bn1, bn1], axis=0)           # (128,2,1)
    mask = np.ones((128, 2, 1), np.float32)
    if h == 0:
        mask[0:T, 0] = 0.0   # x row 0 (half A) = image row -1
    else:
        mask[T:128, 1] = 0.0  # x half-B row 35 = x row 65 = image row 128
    bx, by, xlo, xhi, ylo, yhi = _host_constants(q, h)
    imgs = img_list(q)
    pred_imgs = np.stack([t2_pred[n, ch] for (n, ch) in imgs])
    dsrc = _dsrc_build(pred_imgs, h)
    emat = np.zeros((8, 128), BF)
    for Gi in range(8):
        emat[Gi, 16 * Gi:16 * (Gi + 1)] = 1.0
    return {
        "f1": f1, "f2": f2, "wd": wd, "wf1": wf1, "wf2": wf2,
        "bn": bn, "mask": mask, "bx": bx, "by": by, "ylo": ylo, "yhi": yhi,
        "xlo": xlo, "xhi": xhi, "emat": emat, "dsrc": dsrc.astype(BF),
    }


def unshard_out(raw):
    """raw (2, 2, 128, 1024) f32 -> (19-ish, 64, 256) per-image rows.

    raw[call, slot, 16G+l, 512rh+16jj+m] = out[img(l,slot), 32rh+jj,
    128call+16G+m].
    """
    o = raw.reshape(2, 2, 8, 16, 2, 32, 16)  # call, slot, G, l, rh, jj, m
    full = o.transpose(1, 3, 4, 5, 0, 2, 6).reshape(2, 16, 64, 256)
    return full  # [slot, l, r, col]


def kernel(**inputs):
    from concourse.bass_utils import run_bass_kernel_spmd
    if "nc" not in _NC_CACHE:
        _NC_CACHE["nc"] = build_nc()
    nc = _NC_CACHE["nc"]
    in_maps = [make_inputs(c, **inputs) for c in range(8)]
    res = run_bass_kernel_spmd(nc, in_maps, list(range(8)))
    out = np.zeros((NB, CCLS, H, W), np.float32)
    for c in range(8):
        q, h = c // 2, c % 2
        full = unshard_out(res.results[c]["out"])
        for i, (n, ch) in enumerate(img_list(q)):
            out[n, ch, 64 * h:64 * (h + 1), :] = full[i // 16, i % 16]
    return out


_NC_CACHE = {}
